# revision 20
# baseline (speedup 1.0000x reference)
"""PixelNCELoss Trainium2 kernel.

Strategy (data-parallel, 8 cores = 4 samples x 2 row-shards):
  - Host: gather anchor features Xq/Xk ([950, 256] per sample) from the
    full feature maps using sample_idx, pre-scale Xq by 1/TEMP.
  - Shard each sample's 950 anchor rows over 2 cores: shard 0 = classes
    0..9 (500 rows), shard 1 = classes 10..18 (450 rows). Columns (the
    950 positives/negatives) are permuted per core so the shard's own
    classes come first in the same order as its rows -- this makes the
    diagonal-block mask structure identical on every core, so a single
    SPMD program serves all 8 cores.
  - Device per core: logits = XqT.T @ XkT via fp32r matmul into PSUM,
    with 19 extra one-hot "mask channels" appended to the contraction so
    the same-class block bias (-2^100) is accumulated by the PE itself.
    The positive logit (and its negation) come from a tiny ones-matmul
    over the host-precomputed elementwise product Xq*Xk. Then per row:
    max (negated), exp with bias=-max and fused row-sum, add the
    positive term exp(z_ii - max), log -> loss.
  - Host: reassemble [3800] output.
"""

import numpy as np

TEMP = 0.07
B, C, HW = 4, 256, 128 * 128
NCLS, NV = 19, 50
P = NCLS * NV          # 950
SPLIT = 500            # rows in shard h=0 (classes 0..9); h=1 gets 450
ROWS = (500, 450)
PADR = 512             # padded rows per core (4 tiles of 128)
NT = 4                 # row tiles per core
COL0 = 512             # psum column chunk split (bank-aligned)
COL1 = P - COL0        # 438
MASKVAL = -(2.0 ** 100)
INV_T = 1.0 / TEMP

_cache = {}


def _build_program():
    import concourse.bacc as bacc
    import concourse.tile as tile
    from concourse import mybir

    f32 = mybir.dt.float32
    f32r = mybir.dt.float32r
    AF = mybir.ActivationFunctionType
    ALU = mybir.AluOpType
    AX = mybir.AxisListType

    nc = bacc.Bacc("TRN2", target_bir_lowering=False, debug=False)
    # host pre-packs [p, kc*N + col] layouts so each tensor is ONE DMA
    xq = nc.declare_dram_parameter("xq", [128, 2 * PADR], f32r, isOutput=False)
    xkA = nc.declare_dram_parameter("xkA", [128, 2 * COL0], f32r, isOutput=False)
    xkB = nc.declare_dram_parameter("xkB", [128, 2 * COL1], f32r, isOutput=False)
    nlp = nc.declare_dram_parameter("nlp", [128, NT], f32, isOutput=False)
    xqm = nc.declare_dram_parameter("xqm", [NCLS, PADR], f32r, isOutput=False)
    xkm = nc.declare_dram_parameter("xkm", [NCLS, P], f32r, isOutput=False)
    outp = nc.declare_dram_parameter("out", [128, NT], f32, isOutput=True)

    with tile.TileContext(nc) as tc:
        with (
            tc.tile_pool(name="singles", bufs=1) as singles,
            tc.tile_pool(name="psb", bufs=3, space="PSUM") as psb,
            tc.tile_pool(name="pswarm", bufs=1, space="PSUM") as pswarm,
            tc.tile_pool(name="epool", bufs=2) as epool,
            tc.tile_pool(name="small", bufs=2) as small,
        ):
            # PE warm-up: dummy matmuls on zeroed tiles during the DMA
            # wait, so the real matmuls run at 2.4 GHz (HAM warm).
            # gpsimd memsets run earliest after the start barrier.
            wq = singles.tile([128, 128], f32)
            wk = singles.tile([128, 256], f32)
            nc.gpsimd.memset(wq, 0.0)
            nc.gpsimd.memset(wk, 0.0)
            psw = pswarm.tile([128, 256], f32)
            for i in range(4):
                nc.tensor.matmul(psw, wq, wk, start=(i == 0), stop=(i == 3))
            # dummy Exp: pull the ACT exp-table load off the critical path
            wexp = singles.tile([128, 1], f32)
            nc.scalar.activation(wexp, wq[:, 0:1], AF.Exp, bias=0.0, scale=0.0)

            # DMAs split across both HWDGE engines (SP + Activation)
            xqm_sb = singles.tile([NCLS, PADR], f32r)
            xkm_sb = singles.tile([NCLS, P], f32r)
            nlp_sb = singles.tile([128, NT], f32)
            xq_sb = singles.tile([128, 2, PADR], f32r)
            xkA_sb = singles.tile([128, 2, COL0], f32r)
            xkB_sb = singles.tile([128, 2, COL1], f32r)
            nc.scalar.dma_start(out=xqm_sb, in_=xqm[:, :])
            nc.sync.dma_start(out=xkm_sb, in_=xkm[:, :])
            nc.scalar.dma_start(
                out=xq_sb, in_=xq.rearrange("p (kc m) -> p kc m", kc=2))
            nc.sync.dma_start(
                out=xkA_sb, in_=xkA.rearrange("p (kc m) -> p kc m", kc=2))
            nc.sync.dma_start(
                out=xkB_sb, in_=xkB.rearrange("p (kc m) -> p kc m", kc=2))
            nc.scalar.dma_start(out=nlp_sb, in_=nlp[:, :])

            loss_sb = singles.tile([128, NT], f32)
            negm4 = singles.tile([128, NT], f32)
            earg4 = singles.tile([128, NT], f32)
            S4 = singles.tile([128, NT], f32)

            for t in range(NT):
                r0 = t * 128
                ps = psb.tile([128, P], f32, tag="ps")       # 2 banks
                for kc in range(2):
                    nc.tensor.matmul(ps[:, 0:COL0], xq_sb[:, kc, r0:r0 + 128],
                                     xkA_sb[:, kc, :], start=(kc == 0),
                                     stop=False)
                for kc in range(2):
                    nc.tensor.matmul(ps[:, COL0:P], xq_sb[:, kc, r0:r0 + 128],
                                     xkB_sb[:, kc, :], start=(kc == 0),
                                     stop=False)
                nc.tensor.matmul(ps[:, 0:COL0], xqm_sb[:, r0:r0 + 128],
                                 xkm_sb[:, 0:COL0], start=False, stop=True)
                nc.tensor.matmul(ps[:, COL0:P], xqm_sb[:, r0:r0 + 128],
                                 xkm_sb[:, COL0:P], start=False, stop=True)

                nlpos = nlp_sb[:, t:t + 1]
                nm = small.tile([128, 1], f32, tag="nm")
                nc.vector.reduce_max(nm, ps[:, :], axis=AX.X, negate=True)
                negm = negm4[:, t:t + 1]
                nc.vector.tensor_tensor(out=negm, in0=nm, in1=nlpos,
                                        op=ALU.min)
                nc.vector.tensor_sub(earg4[:, t:t + 1], negm, nlpos)

                E = epool.tile([128, P], f32, tag="E")
                nc.scalar.activation(E, ps[:, :], AF.Exp, bias=negm,
                                     scale=1.0, accum_out=S4[:, t:t + 1])

            # batched epilogue: one Exp, one Ln for the whole kernel
            e_d4 = singles.tile([128, NT], f32)
            nc.scalar.activation(e_d4, earg4, AF.Exp, bias=0.0, scale=1.0)
            Sf = singles.tile([128, NT], f32)
            nc.vector.tensor_add(Sf, S4, e_d4)
            logS = singles.tile([128, NT], f32)
            nc.scalar.activation(logS, Sf, AF.Ln)
            nc.vector.tensor_sub(loss_sb, logS, earg4)

            nc.sync.dma_start(out=outp[:, :], in_=loss_sb)

    nc.compile()
    return nc


def _get_program():
    if "nc" not in _cache:
        _cache["nc"] = _build_program()
    return _cache["nc"]


def _host_inputs(feats_q, feats_k, sample_idx):
    """Build the 8 per-core input maps."""
    q = np.ascontiguousarray(feats_q, dtype=np.float32).reshape(B, C, HW)
    k = np.ascontiguousarray(feats_k, dtype=np.float32).reshape(B, C, HW)
    idx = np.asarray(sample_idx).reshape(B, P)

    # mask channels: row side carries MASKVAL one-hot of the row's local
    # class, col side carries the plain one-hot -> PE accumulates
    # MASKVAL wherever row and col share a class (incl. the diagonal)
    cls_row = np.arange(PADR) // NV            # local class per row
    cls_col = np.arange(P) // NV
    xqm = np.zeros((NCLS, PADR), dtype=np.float32)
    xkm = np.zeros((NCLS, P), dtype=np.float32)
    for cl in range(NCLS):
        xqm[cl, cls_row == cl] = MASKVAL
        xkm[cl, cls_col == cl] = 1.0

    colperm1 = np.concatenate([np.arange(SPLIT, P), np.arange(0, SPLIT)])

    in_maps = []
    for b in range(B):
        XqT = q[b][:, idx[b]]                     # [C, P]
        XkT = k[b][:, idx[b]]
        for h in range(2):
            r0g = 0 if h == 0 else SPLIT
            nrows = ROWS[h]
            xq = np.zeros((C, PADR), dtype=np.float32)
            xq[:, :nrows] = XqT[:, r0g:r0g + nrows] * np.float32(INV_T)
            xk = XkT if h == 0 else XkT[:, colperm1]
            # negated positive logits, exact (f64 accumulate), laid [p, t]
            lp = np.zeros(PADR, dtype=np.float32)
            lp[:nrows] = -np.einsum(
                "ci,ci->i", xq[:, :nrows].astype(np.float64),
                xk[:, :nrows].astype(np.float64)).astype(np.float32)
            nlp4 = np.ascontiguousarray(lp.reshape(NT, 128).T)
            # pack [p, kc*N + col]: row kc*128+p of the [C, N] matrix
            xq_p = np.ascontiguousarray(
                xq.reshape(2, 128, PADR).transpose(1, 0, 2).reshape(
                    128, 2 * PADR))
            xkA_p = np.ascontiguousarray(
                xk[:, 0:COL0].reshape(2, 128, COL0).transpose(1, 0, 2).reshape(
                    128, 2 * COL0))
            xkB_p = np.ascontiguousarray(
                xk[:, COL0:P].reshape(2, 128, COL1).transpose(1, 0, 2).reshape(
                    128, 2 * COL1))
            in_maps.append({
                "xq": xq_p,
                "xkA": xkA_p,
                "xkB": xkB_p,
                "nlp": nlp4,
                "xqm": xqm,
                "xkm": xkm,
            })
    return in_maps


def _assemble(results):
    out = np.zeros((B, P), dtype=np.float32)
    for b in range(B):
        for h in range(2):
            r0g = 0 if h == 0 else SPLIT
            nrows = ROWS[h]
            arr = np.asarray(results[2 * b + h]["out"])  # [128, NT]
            loss = arr.T.reshape(PADR)                   # index t*128+p
            out[b, r0g:r0g + nrows] = loss[:nrows]
    return out.reshape(-1)


def kernel(feats_q, feats_k, sample_idx):
    from concourse.bass_utils import run_bass_kernel_spmd

    nc = _get_program()
    in_maps = _host_inputs(feats_q, feats_k, sample_idx)
    res = run_bass_kernel_spmd(nc, in_maps, list(range(8)))
    return _assemble(res.results)


# revision 21
# speedup vs baseline: 1.2711x; 1.2711x over previous
"""PixelNCELoss Trainium2 kernel.

Strategy (data-parallel, 8 cores = 4 samples x 2 row-shards):
  - Host: gather anchor features Xq/Xk ([950, 256] per sample) from the
    full feature maps using sample_idx, pre-scale Xq by 1/TEMP.
  - Shard each sample's 950 anchor rows over 2 cores: shard 0 = classes
    0..9 (500 rows), shard 1 = classes 10..18 (450 rows). Columns (the
    950 positives/negatives) are permuted per core so the shard's own
    classes come first in the same order as its rows -- this makes the
    diagonal-block mask structure identical on every core, so a single
    SPMD program serves all 8 cores.
  - Device per core: logits = XqT.T @ XkT via fp32r matmul into PSUM,
    with 19 extra one-hot "mask channels" appended to the contraction so
    the same-class block bias (-2^100) is accumulated by the PE itself.
    The positive logit (and its negation) come from a tiny ones-matmul
    over the host-precomputed elementwise product Xq*Xk. Then per row:
    max (negated), exp with bias=-max and fused row-sum, add the
    positive term exp(z_ii - max), log -> loss.
  - Host: reassemble [3800] output.
"""

import numpy as np

TEMP = 0.07
B, C, HW = 4, 256, 128 * 128
NCLS, NV = 19, 50
P = NCLS * NV          # 950
SPLIT = 500            # rows in shard h=0 (classes 0..9); h=1 gets 450
ROWS = (500, 450)
PADR = 512             # padded rows per core (4 tiles of 128)
NT = 4                 # row tiles per core
COL0 = 512             # psum column chunk split (bank-aligned)
COL1 = P - COL0        # 438
MASKVAL = -20480.0   # exact in fp16; dominates |logits| <= ~2000
INV_T = 1.0 / TEMP

_cache = {}


def _build_program():
    import concourse.bacc as bacc
    import concourse.tile as tile
    from concourse import mybir

    f32 = mybir.dt.float32
    f16 = mybir.dt.float16
    AF = mybir.ActivationFunctionType
    ALU = mybir.AluOpType
    AX = mybir.AxisListType

    nc = bacc.Bacc("TRN2", target_bir_lowering=False, debug=False)
    # host pre-packs [p, kc*N + col] layouts so each tensor is ONE DMA
    xq = nc.declare_dram_parameter("xq", [128, 2 * PADR], f16, isOutput=False)
    xkA = nc.declare_dram_parameter("xkA", [128, 2 * COL0], f16, isOutput=False)
    xkB = nc.declare_dram_parameter("xkB", [128, 2 * COL1], f16, isOutput=False)
    nlp = nc.declare_dram_parameter("nlp", [128, NT], f32, isOutput=False)
    xqm = nc.declare_dram_parameter("xqm", [NCLS, PADR], f16, isOutput=False)
    xkm = nc.declare_dram_parameter("xkm", [NCLS, P], f16, isOutput=False)
    outp = nc.declare_dram_parameter("out", [128, NT], f32, isOutput=True)

    with tile.TileContext(nc) as tc:
        with (
            tc.tile_pool(name="singles", bufs=1) as singles,
            tc.tile_pool(name="psb", bufs=3, space="PSUM") as psb,
            tc.tile_pool(name="pswarm", bufs=1, space="PSUM") as pswarm,
            tc.tile_pool(name="epool", bufs=2) as epool,
            tc.tile_pool(name="small", bufs=2) as small,
        ):
            # PE warm-up: dummy matmuls on zeroed tiles during the DMA
            # wait, so the real matmuls run at 2.4 GHz (HAM warm).
            # gpsimd memsets run earliest after the start barrier.
            wq = singles.tile([128, 128], f32)
            wk = singles.tile([128, 256], f32)
            nc.gpsimd.memset(wq, 0.0)
            nc.gpsimd.memset(wk, 0.0)
            psw = pswarm.tile([128, 256], f32)
            for i in range(4):
                nc.tensor.matmul(psw, wq, wk, start=(i == 0), stop=(i == 3))
            # dummy Exp: pull the ACT exp-table load off the critical path
            wexp = singles.tile([128, 1], f32)
            nc.scalar.activation(wexp, wq[:, 0:1], AF.Exp, bias=0.0, scale=0.0)

            # DMAs split across both HWDGE engines (SP + Activation)
            xqm_sb = singles.tile([NCLS, PADR], f16)
            xkm_sb = singles.tile([NCLS, P], f16)
            nlp_sb = singles.tile([128, NT], f32)
            xq_sb = singles.tile([128, 2, PADR], f16)
            xkA_sb = singles.tile([128, 2, COL0], f16)
            xkB_sb = singles.tile([128, 2, COL1], f16)
            nc.sync.dma_start(out=xqm_sb, in_=xqm[:, :])
            nc.sync.dma_start(out=xkm_sb, in_=xkm[:, :])
            nc.sync.dma_start(
                out=xq_sb, in_=xq.rearrange("p (kc m) -> p kc m", kc=2))
            nc.sync.dma_start(
                out=xkA_sb, in_=xkA.rearrange("p (kc m) -> p kc m", kc=2))
            nc.sync.dma_start(
                out=xkB_sb, in_=xkB.rearrange("p (kc m) -> p kc m", kc=2))
            nc.sync.dma_start(out=nlp_sb, in_=nlp[:, :])

            loss_sb = singles.tile([128, NT], f32)
            negm4 = singles.tile([128, NT], f32)
            earg4 = singles.tile([128, NT], f32)
            S4 = singles.tile([128, NT], f32)

            for t in range(NT):
                r0 = t * 128
                ps = psb.tile([128, P], f32, tag="ps")       # 2 banks
                for kc in range(2):
                    nc.tensor.matmul(ps[:, 0:COL0], xq_sb[:, kc, r0:r0 + 128],
                                     xkA_sb[:, kc, :], start=(kc == 0),
                                     stop=False)
                for kc in range(2):
                    nc.tensor.matmul(ps[:, COL0:P], xq_sb[:, kc, r0:r0 + 128],
                                     xkB_sb[:, kc, :], start=(kc == 0),
                                     stop=False)
                nc.tensor.matmul(ps[:, 0:COL0], xqm_sb[:, r0:r0 + 128],
                                 xkm_sb[:, 0:COL0], start=False, stop=True)
                nc.tensor.matmul(ps[:, COL0:P], xqm_sb[:, r0:r0 + 128],
                                 xkm_sb[:, COL0:P], start=False, stop=True)

                nlpos = nlp_sb[:, t:t + 1]
                nm = small.tile([128, 1], f32, tag="nm")
                nc.vector.reduce_max(nm, ps[:, :], axis=AX.X, negate=True)
                negm = negm4[:, t:t + 1]
                nc.vector.tensor_tensor(out=negm, in0=nm, in1=nlpos,
                                        op=ALU.min)
                nc.vector.tensor_sub(earg4[:, t:t + 1], negm, nlpos)

                E = epool.tile([128, P], f32, tag="E")
                nc.scalar.activation(E, ps[:, :], AF.Exp, bias=negm,
                                     scale=1.0, accum_out=S4[:, t:t + 1])

            # batched epilogue: one Exp, one Ln for the whole kernel
            e_d4 = singles.tile([128, NT], f32)
            nc.scalar.activation(e_d4, earg4, AF.Exp, bias=0.0, scale=1.0)
            Sf = singles.tile([128, NT], f32)
            nc.vector.tensor_add(Sf, S4, e_d4)
            logS = singles.tile([128, NT], f32)
            nc.scalar.activation(logS, Sf, AF.Ln)
            nc.vector.tensor_sub(loss_sb, logS, earg4)

            nc.sync.dma_start(out=outp[:, :], in_=loss_sb)

    nc.compile()
    return nc


def _get_program():
    if "nc" not in _cache:
        _cache["nc"] = _build_program()
    return _cache["nc"]


def _host_inputs(feats_q, feats_k, sample_idx):
    """Build the 8 per-core input maps."""
    q = np.ascontiguousarray(feats_q, dtype=np.float32).reshape(B, C, HW)
    k = np.ascontiguousarray(feats_k, dtype=np.float32).reshape(B, C, HW)
    idx = np.asarray(sample_idx).reshape(B, P)

    # mask channels: row side carries MASKVAL one-hot of the row's local
    # class, col side carries the plain one-hot -> PE accumulates
    # MASKVAL wherever row and col share a class (incl. the diagonal)
    cls_row = np.arange(PADR) // NV            # local class per row
    cls_col = np.arange(P) // NV
    xqm = np.zeros((NCLS, PADR), dtype=np.float16)
    xkm = np.zeros((NCLS, P), dtype=np.float16)
    for cl in range(NCLS):
        xqm[cl, cls_row == cl] = MASKVAL
        xkm[cl, cls_col == cl] = 1.0

    colperm1 = np.concatenate([np.arange(SPLIT, P), np.arange(0, SPLIT)])

    in_maps = []
    for b in range(B):
        XqT = q[b][:, idx[b]]                     # [C, P]
        XkT = k[b][:, idx[b]]
        for h in range(2):
            r0g = 0 if h == 0 else SPLIT
            nrows = ROWS[h]
            xq = np.zeros((C, PADR), dtype=np.float32)
            xq[:, :nrows] = XqT[:, r0g:r0g + nrows] * np.float32(INV_T)
            xk = XkT if h == 0 else XkT[:, colperm1]
            # negated positive logits, exact (f64 accumulate), laid [p, t]
            lp = np.zeros(PADR, dtype=np.float32)
            lp[:nrows] = -np.einsum(
                "ci,ci->i", xq[:, :nrows].astype(np.float64),
                xk[:, :nrows].astype(np.float64)).astype(np.float32)
            nlp4 = np.ascontiguousarray(lp.reshape(NT, 128).T)
            # pack [p, kc*N + col]: row kc*128+p of the [C, N] matrix
            xq_p = np.ascontiguousarray(
                xq.reshape(2, 128, PADR).transpose(1, 0, 2).reshape(
                    128, 2 * PADR))
            xkA_p = np.ascontiguousarray(
                xk[:, 0:COL0].reshape(2, 128, COL0).transpose(1, 0, 2).reshape(
                    128, 2 * COL0))
            xkB_p = np.ascontiguousarray(
                xk[:, COL0:P].reshape(2, 128, COL1).transpose(1, 0, 2).reshape(
                    128, 2 * COL1))
            in_maps.append({
                "xq": xq_p.astype(np.float16),
                "xkA": xkA_p.astype(np.float16),
                "xkB": xkB_p.astype(np.float16),
                "nlp": nlp4,
                "xqm": xqm,
                "xkm": xkm,
            })
    return in_maps


def _assemble(results):
    out = np.zeros((B, P), dtype=np.float32)
    for b in range(B):
        for h in range(2):
            r0g = 0 if h == 0 else SPLIT
            nrows = ROWS[h]
            arr = np.asarray(results[2 * b + h]["out"])  # [128, NT]
            loss = arr.T.reshape(PADR)                   # index t*128+p
            out[b, r0g:r0g + nrows] = loss[:nrows]
    return out.reshape(-1)


def kernel(feats_q, feats_k, sample_idx):
    from concourse.bass_utils import run_bass_kernel_spmd

    nc = _get_program()
    in_maps = _host_inputs(feats_q, feats_k, sample_idx)
    res = run_bass_kernel_spmd(nc, in_maps, list(range(8)))
    return _assemble(res.results)
